# revision 2
# baseline (speedup 1.0000x reference)
"""Trainium2 Bass kernel: single-head attention (projections + masked softmax),
data-parallel over batch across 8 NeuronCores.

Per-core dataflow (one batch element per core):
  q/k/v [L, 1024] f32 --SWDGE cast-DMA--> bf16 SBUF [128, 1024] tiles
    --xbar DMA transpose--> [128, 8, 128] transposed blocks (dword on partitions)
  projections accumulate in PSUM (contraction over dword chunks):
    qsT/ksT [d_k=128, L] (copied to SBUF as float32r for precise scores)
    vs [L-chunk, d_v]  -> masked vs_aug [128, lt, 129] bf16 (ones-column = mask)
  scores S^T [LK-chunk, LQ-block] = ksT_chunk.T @ qsT_block (f32r, full PE rate)
  exp fused with 1/temperature scaling on ScalarE -> bf16
  out_aug [LQ-chunk, 129] += expS^T.T @ vs_aug   (accumulated over LK chunks;
    column 128 accumulates the softmax denominator via the mask column)
  normalize: out = out_aug[:, :128] * reciprocal(out_aug[:, 128])
"""
import numpy as np

B, LQ, LK, DW, DK, DV = 8, 2048, 2048, 1024, 128, 128
TEMPERATURE = 11.313708498984761
N_CORES = 8
P = 128


def build(lq=LQ, lk=LK, dw=DW, dk=DK, dv=DV, lqb=512):
    import concourse.tile as tile
    import concourse.mybir as mybir
    from concourse import bacc

    nc = bacc.Bacc("TRN2", target_bir_lowering=False, debug=False,
                   num_devices=N_CORES)
    dt = mybir.dt
    f32, bf16, f32r, i32 = dt.float32, dt.bfloat16, dt.float32r, dt.int32
    NC = dw // P
    LQt, LKt = lq // P, lk // P
    NBLK = lq // lqb
    C4 = lqb // P

    q = nc.declare_dram_parameter("q", [lq, dw], f32, isOutput=False)
    k = nc.declare_dram_parameter("k", [lk, dw], f32, isOutput=False)
    v = nc.declare_dram_parameter("v", [lk, dw], f32, isOutput=False)
    ml = nc.declare_dram_parameter("ml", [P, 1], i32, isOutput=False)
    wq = nc.declare_dram_parameter("wq", [dw, dk], f32, isOutput=False)
    wk = nc.declare_dram_parameter("wk", [dw, dk], f32, isOutput=False)
    wv = nc.declare_dram_parameter("wv", [dw, dv], f32, isOutput=False)
    out = nc.declare_dram_parameter("out", [lq, dv], f32, isOutput=True)

    with tile.TileContext(nc) as tc:
        with tc.tile_pool(name="sb", bufs=1) as sb, \
             tc.tile_pool(name="ps", bufs=1, space="PSUM") as ps:
            # sequence mask: mask[p, lt] = (lt*128 + p) < memory_length
            iota = sb.tile([P, LKt], i32, tag="iota")
            nc.gpsimd.iota(iota[:], pattern=[[P, LKt]], base=0,
                           channel_multiplier=1)
            mlt = sb.tile([P, 1], i32, tag="mlt")
            nc.gpsimd.dma_start(mlt[:], ml[:])
            mask = sb.tile([P, LKt], f32, tag="mask")
            nc.vector.tensor_tensor(mask[:], iota[:],
                                    mlt[:].to_broadcast([P, LKt]),
                                    mybir.AluOpType.is_lt)

            wts = {}
            for nm, src in (("wq", wq), ("wk", wk), ("wv", wv)):
                w = sb.tile([P, NC, dk], bf16, tag=nm, name=nm + "_sb")
                nc.gpsimd.dma_start(w[:], src.rearrange("(c p) d -> p c d", p=P))
                wts[nm] = w

            qsT = sb.tile([P, lq], f32r, tag="qsT")
            ksT = sb.tile([P, lk], f32r, tag="ksT")
            vsaug = sb.tile([P, LKt, dv + 1], bf16, tag="vsaug")

            # q, k projections -> qsT / ksT
            for nm, src, dst, L_t in (("wq", q, qsT, LQt), ("wk", k, ksT, LKt)):
                w = wts[nm]
                pst = [ps.tile([P, 4 * P], f32, tag="pbank", bufs=8,
                               name=f"ps_{nm}_{s}") for s in range(L_t // 4)]
                for lt in range(L_t):
                    ld = sb.tile([P, dw], bf16, tag="ld", bufs=4,
                                 name=f"ld_{nm}_{lt}")
                    nc.gpsimd.dma_start(ld[:], src[lt * P:(lt + 1) * P, :])
                    tb = sb.tile([P, NC, P], bf16, tag="tblk", bufs=4,
                                 name=f"tb_{nm}_{lt}")
                    nc.sync.dma_start_transpose(tb[:], ld[:])
                    po = pst[lt // 4][:, (lt % 4) * P:(lt % 4 + 1) * P]
                    for c in range(NC):
                        nc.tensor.matmul(po, w[:, c, :], tb[:, c, :],
                                         start=(c == 0), stop=(c == NC - 1))
                for s in range(L_t // 4):
                    nc.vector.tensor_copy(dst[:, s * 4 * P:(s + 1) * 4 * P],
                                          pst[s][:])

            # v projection -> masked vs_aug (ones column = mask column)
            w = wts["wv"]
            pvt = [ps.tile([P, 4 * P], f32, tag="pbank", bufs=8,
                           name=f"ps_v_{s}") for s in range(LKt // 4)]
            for lt in range(LKt):
                ld = sb.tile([P, dw], bf16, tag="ld", bufs=4, name=f"ld_v_{lt}")
                nc.gpsimd.dma_start(ld[:], v[lt * P:(lt + 1) * P, :])
                tb = sb.tile([P, NC, P], bf16, tag="tblk", bufs=4,
                             name=f"tb_v_{lt}")
                nc.sync.dma_start_transpose(tb[:], ld[:])
                po = pvt[lt // 4][:, (lt % 4) * P:(lt % 4 + 1) * P]
                for c in range(NC):
                    nc.tensor.matmul(po, tb[:, c, :], w[:, c, :],
                                     start=(c == 0), stop=(c == NC - 1))
                nc.vector.tensor_scalar(vsaug[:, lt, :dv], po,
                                        mask[:, lt:lt + 1], None,
                                        mybir.AluOpType.mult)
                nc.vector.tensor_copy(vsaug[:, lt, dv:dv + 1],
                                      mask[:, lt:lt + 1])

            # scores + softmax + AV, one LQ block at a time
            inv_t = 1.0 / TEMPERATURE
            for blk in range(NBLK):
                avp = [ps.tile([P, dv + 1], f32, tag="pbank", bufs=8,
                               name=f"av_{blk}_{c4}") for c4 in range(C4)]
                for j in range(LKt):
                    sps = ps.tile([P, lqb], f32, tag="pbank", bufs=8,
                                  name=f"sps_{blk}_{j}")
                    nc.tensor.matmul(sps[:], ksT[:, j * P:(j + 1) * P],
                                     qsT[:, blk * lqb:(blk + 1) * lqb],
                                     start=True, stop=True)
                    es = sb.tile([P, lqb], bf16, tag="es", bufs=3,
                                 name=f"es_{blk}_{j}")
                    nc.scalar.activation(es[:], sps[:],
                                         mybir.ActivationFunctionType.Exp,
                                         scale=inv_t)
                    for c4 in range(C4):
                        nc.tensor.matmul(avp[c4][:], es[:, c4 * P:(c4 + 1) * P],
                                         vsaug[:, j, :],
                                         start=(j == 0), stop=(j == LKt - 1))
                osb = sb.tile([P, C4, dv], f32, tag="osb", bufs=2,
                              name=f"osb_{blk}")
                for c4 in range(C4):
                    rec = sb.tile([P, 1], f32, tag="rec", bufs=4,
                                  name=f"rec_{blk}_{c4}")
                    nc.vector.reciprocal(rec[:], avp[c4][:, dv:dv + 1])
                    nc.vector.tensor_scalar(osb[:, c4, :], avp[c4][:, :dv],
                                            rec[:], None,
                                            mybir.AluOpType.mult)
                nc.sync.dma_start(
                    out.rearrange("(b c p) d -> b p c d", c=C4, p=P)[blk],
                    osb[:])
    nc.compile()
    return nc


_built = None


def _get_built():
    global _built
    if _built is None:
        _built = build()
    return _built


def make_in_maps(q, k, v, memory_lengths, Wq, Wk, Wv):
    q = np.asarray(q, dtype=np.float32)
    k = np.asarray(k, dtype=np.float32)
    v = np.asarray(v, dtype=np.float32)
    ml = np.asarray(memory_lengths, dtype=np.int32)
    Wq = np.asarray(Wq, dtype=np.float32)
    Wk = np.asarray(Wk, dtype=np.float32)
    Wv = np.asarray(Wv, dtype=np.float32)
    return [
        {"q": q[b], "k": k[b], "v": v[b],
         "ml": np.full((P, 1), ml[b], dtype=np.int32),
         "wq": Wq, "wk": Wk, "wv": Wv}
        for b in range(B)
    ]


def kernel(q, k, v, memory_lengths, Wq, Wk, Wv):
    from concourse.bass_utils import run_bass_kernel_spmd
    nc = _get_built()
    in_maps = make_in_maps(q, k, v, memory_lengths, Wq, Wk, Wv)
    res = run_bass_kernel_spmd(nc, in_maps, core_ids=list(range(N_CORES)))
    return np.stack([res.results[b]["out"] for b in range(B)]).astype(np.float32)


if __name__ == "__main__":
    d = np.load("/root/problem/ref_cache.npz")
    outp = kernel(d["q"], d["k"], d["v"], d["memory_lengths"],
                  d["Wq"], d["Wk"], d["Wv"])
    exp = d["expected"]
    err = np.linalg.norm(outp - exp) / np.linalg.norm(exp)
    print("Relative error:", err)


# revision 4
# speedup vs baseline: 238.2110x; 238.2110x over previous
"""Trainium2 Bass kernel: single-head attention (projections + masked softmax),
data-parallel over batch across 8 NeuronCores.

Per-core dataflow (one batch element per core):
  q/k/v [L, 1024] f32 --SWDGE cast-DMA--> bf16 SBUF [128, 1024] tiles
    --xbar DMA transpose--> [128, 8, 128] transposed blocks (dword on partitions)
  projections accumulate in PSUM (contraction over dword chunks):
    qsT/ksT [d_k=128, L] (copied to SBUF as float32r for precise scores)
    vs [L-chunk, d_v]  -> masked vs_aug [128, lt, 129] bf16 (ones-column = mask)
  scores S^T [LK-chunk, LQ-block] = ksT_chunk.T @ qsT_block (f32r, full PE rate)
  exp fused with 1/temperature scaling on ScalarE -> bf16
  out_aug [LQ-chunk, 129] += expS^T.T @ vs_aug   (accumulated over LK chunks;
    column 128 accumulates the softmax denominator via the mask column)
  normalize: out = out_aug[:, :128] * reciprocal(out_aug[:, 128])
"""
import numpy as np

B, LQ, LK, DW, DK, DV = 8, 2048, 2048, 1024, 128, 128
TEMPERATURE = 11.313708498984761
N_CORES = 8
P = 128


def build(lq=LQ, lk=LK, dw=DW, dk=DK, dv=DV, lqb=512, repeat=1):
    import contextlib
    import concourse.tile as tile
    import concourse.mybir as mybir
    from concourse import bacc

    nc = bacc.Bacc("TRN2", target_bir_lowering=False, debug=False,
                   num_devices=N_CORES)
    dt = mybir.dt
    f32, bf16, f32r, i32 = dt.float32, dt.bfloat16, dt.float32r, dt.int32
    NC = dw // P
    LQt, LKt = lq // P, lk // P
    NBLK = lq // lqb
    C4 = lqb // P

    q = nc.declare_dram_parameter("q", [lq, dw], f32, isOutput=False)
    k = nc.declare_dram_parameter("k", [lk, dw], f32, isOutput=False)
    v = nc.declare_dram_parameter("v", [lk, dw], f32, isOutput=False)
    ml = nc.declare_dram_parameter("ml", [P, 1], i32, isOutput=False)
    wq = nc.declare_dram_parameter("wq", [dw, dk], f32, isOutput=False)
    wk = nc.declare_dram_parameter("wk", [dw, dk], f32, isOutput=False)
    wv = nc.declare_dram_parameter("wv", [dw, dv], f32, isOutput=False)
    out = nc.declare_dram_parameter("out", [lq, dv], f32, isOutput=True)

    with tile.TileContext(nc) as tc:
        rep_ctx = (tc.For_i(0, repeat, 1, hint_engines=(mybir.EngineType.PE,))
                   if repeat > 1 else contextlib.nullcontext())
        with rep_ctx, \
             tc.tile_pool(name="sb", bufs=1) as sb, \
             tc.tile_pool(name="ps", bufs=1, space="PSUM") as ps:
            # sequence mask: mask[p, lt] = (lt*128 + p) < memory_length
            iota = sb.tile([P, LKt], i32, tag="iota")
            nc.gpsimd.iota(iota[:], pattern=[[P, LKt]], base=0,
                           channel_multiplier=1)
            mlt = sb.tile([P, 1], i32, tag="mlt")
            nc.gpsimd.dma_start(mlt[:], ml[:])
            mask = sb.tile([P, LKt], f32, tag="mask")
            nc.vector.tensor_tensor(mask[:], iota[:],
                                    mlt[:].to_broadcast([P, LKt]),
                                    mybir.AluOpType.is_lt)

            wts = {}
            for nm, src in (("wq", wq), ("wk", wk), ("wv", wv)):
                w = sb.tile([P, NC, dk], bf16, tag=nm, name=nm + "_sb")
                nc.gpsimd.dma_start(w[:], src.rearrange("(c p) d -> p c d", p=P))
                wts[nm] = w

            qsT = sb.tile([P, lq], f32r, tag="qsT")
            ksT = sb.tile([P, lk], f32r, tag="ksT")
            vsaug = sb.tile([P, LKt, dv + 1], bf16, tag="vsaug")

            # q, k projections -> qsT / ksT
            for nm, src, dst, L_t in (("wq", q, qsT, LQt), ("wk", k, ksT, LKt)):
                w = wts[nm]
                pst = [ps.tile([P, 4 * P], f32, tag="pbank", bufs=8,
                               name=f"ps_{nm}_{s}") for s in range(L_t // 4)]
                for lt in range(L_t):
                    ld = sb.tile([P, dw], bf16, tag="ld", bufs=4,
                                 name=f"ld_{nm}_{lt}")
                    nc.gpsimd.dma_start(ld[:], src[lt * P:(lt + 1) * P, :])
                    tb = sb.tile([P, NC, P], bf16, tag="tblk", bufs=4,
                                 name=f"tb_{nm}_{lt}")
                    nc.sync.dma_start_transpose(tb[:], ld[:])
                    po = pst[lt // 4][:, (lt % 4) * P:(lt % 4 + 1) * P]
                    for c in range(NC):
                        nc.tensor.matmul(po, w[:, c, :], tb[:, c, :],
                                         start=(c == 0), stop=(c == NC - 1))
                for s in range(L_t // 4):
                    nc.vector.tensor_copy(dst[:, s * 4 * P:(s + 1) * 4 * P],
                                          pst[s][:])

            # v projection -> masked vs_aug (ones column = mask column)
            w = wts["wv"]
            pvt = [ps.tile([P, 4 * P], f32, tag="pbank", bufs=8,
                           name=f"ps_v_{s}") for s in range(LKt // 4)]
            for lt in range(LKt):
                ld = sb.tile([P, dw], bf16, tag="ld", bufs=4, name=f"ld_v_{lt}")
                nc.gpsimd.dma_start(ld[:], v[lt * P:(lt + 1) * P, :])
                tb = sb.tile([P, NC, P], bf16, tag="tblk", bufs=4,
                             name=f"tb_v_{lt}")
                nc.sync.dma_start_transpose(tb[:], ld[:])
                po = pvt[lt // 4][:, (lt % 4) * P:(lt % 4 + 1) * P]
                for c in range(NC):
                    nc.tensor.matmul(po, tb[:, c, :], w[:, c, :],
                                     start=(c == 0), stop=(c == NC - 1))
                nc.vector.tensor_scalar(vsaug[:, lt, :dv], po,
                                        mask[:, lt:lt + 1], None,
                                        mybir.AluOpType.mult)
                nc.vector.tensor_copy(vsaug[:, lt, dv:dv + 1],
                                      mask[:, lt:lt + 1])

            # scores + softmax + AV, one LQ block at a time
            inv_t = 1.0 / TEMPERATURE
            for blk in range(NBLK):
                avp = [ps.tile([P, dv + 1], f32, tag="pbank", bufs=8,
                               name=f"av_{blk}_{c4}") for c4 in range(C4)]
                for j in range(LKt):
                    sps = ps.tile([P, lqb], f32, tag="pbank", bufs=8,
                                  name=f"sps_{blk}_{j}")
                    nc.tensor.matmul(sps[:], ksT[:, j * P:(j + 1) * P],
                                     qsT[:, blk * lqb:(blk + 1) * lqb],
                                     start=True, stop=True)
                    es = sb.tile([P, lqb], bf16, tag="es", bufs=3,
                                 name=f"es_{blk}_{j}")
                    nc.scalar.activation(es[:], sps[:],
                                         mybir.ActivationFunctionType.Exp,
                                         scale=inv_t)
                    for c4 in range(C4):
                        nc.tensor.matmul(avp[c4][:], es[:, c4 * P:(c4 + 1) * P],
                                         vsaug[:, j, :],
                                         start=(j == 0), stop=(j == LKt - 1))
                osb = sb.tile([P, C4, dv], f32, tag="osb", bufs=2,
                              name=f"osb_{blk}")
                for c4 in range(C4):
                    rec = sb.tile([P, 1], f32, tag="rec", bufs=4,
                                  name=f"rec_{blk}_{c4}")
                    nc.vector.reciprocal(rec[:], avp[c4][:, dv:dv + 1])
                    nc.vector.tensor_scalar(osb[:, c4, :], avp[c4][:, :dv],
                                            rec[:], None,
                                            mybir.AluOpType.mult)
                nc.sync.dma_start(
                    out.rearrange("(b c p) d -> b p c d", c=C4, p=P)[blk],
                    osb[:])
    nc.compile()
    return nc


_built = None


def _get_built():
    global _built
    if _built is None:
        _built = build()
    return _built


def make_in_maps(q, k, v, memory_lengths, Wq, Wk, Wv):
    q = np.asarray(q, dtype=np.float32)
    k = np.asarray(k, dtype=np.float32)
    v = np.asarray(v, dtype=np.float32)
    ml = np.asarray(memory_lengths, dtype=np.int32)
    Wq = np.asarray(Wq, dtype=np.float32)
    Wk = np.asarray(Wk, dtype=np.float32)
    Wv = np.asarray(Wv, dtype=np.float32)
    return [
        {"q": q[b], "k": k[b], "v": v[b],
         "ml": np.full((P, 1), ml[b], dtype=np.int32),
         "wq": Wq, "wk": Wk, "wv": Wv}
        for b in range(B)
    ]


def kernel(q, k, v, memory_lengths, Wq, Wk, Wv):
    from concourse.bass_utils import run_bass_kernel_spmd
    nc = _get_built()
    in_maps = make_in_maps(q, k, v, memory_lengths, Wq, Wk, Wv)
    res = run_bass_kernel_spmd(nc, in_maps, core_ids=list(range(N_CORES)))
    return np.stack([res.results[b]["out"] for b in range(B)]).astype(np.float32)


if __name__ == "__main__":
    d = np.load("/root/problem/ref_cache.npz")
    outp = kernel(d["q"], d["k"], d["v"], d["memory_lengths"],
                  d["Wq"], d["Wk"], d["Wv"])
    exp = d["expected"]
    err = np.linalg.norm(outp - exp) / np.linalg.norm(exp)
    print("Relative error:", err)
